# revision 35
# baseline (speedup 1.0000x reference)
"""Trainium2 Bass kernel for nn_Attention (B=4, T=1024, C=1024, 16 heads).

Sharding: 8 cores = (batch b, head-group g). Core i = 2b+g handles heads
[8g, 8g+8) of batch b for ALL 1024 tokens — tensor-parallel over heads,
column-parallel Wproj. Zero redundant FLOPs (512 matmuls/core vs 648 for
the old batch×query-half scheme, which recomputed K/V for the whole
batch on both cores of a pair). Each core emits the partial
y^T = Wproj[:, cols g] @ out^T_g for all tokens; the host unshard sums
the two partials per batch (a 4M-element add, same order as the
transpose it already does).

Everything on-chip is laid out so no transposes are ever needed:
  - the host passes x^T; Q^T/K^T come out of their projections in
    [chan, tok] layout directly
  - V is produced in [tok, chan] layout with a leading ones-column per
    head, so the PV matmul's PSUM row 0 is the softmax denominator Z
    (row 0 is the only base the custom-DVE reciprocal and GpSimd
    partition_broadcast support)
  - softmax runs on S^T (keys on partitions) with no max-subtraction
    (logits are O(6), exp is safe); one [128,1024] exp covers both
    query halves of a head
  - normalization: fast-DVE reciprocal of the Z row, partition
    broadcast, one DVE multiply, and a SBUF->SBUF DMA shifts rows 1:65
    into the out^T tile

All matmul operands are bf16 (same 1 cycle/row PE rate as float32r but
half the HBM/DMA traffic; an fp32r A/B measured 238us vs 174us for the
same structure). PSUM accumulation is fp32. Rel err vs the fp32
reference ~8e-3 (gate 2e-2).

Schedule: the ScalarE engine is ~50% busy with 64 [128,1024] EXPs
(1.09us each), and the in-order PE queue would stall on them, so every
EXP-gated PV chain is preceded by ~3.6us of EXP-independent matmul
work: S of both heads first, then the next pair's K^T projection
between, then PV of head a, then the next pair's Q^T projection, then
PV of head b. Warm-up matmuls on a memset ones tile ramp the PE
p-state during the initial x^T DMA (the PE drops to 1.2/0.65 GHz after
any >~1us idle gap and takes ~3us of continuous work to re-reach
2.4 GHz); pair 3 uses warm fills and the first Y tiles' kc0..2 partial
contractions to bridge the final norm chains, so the output projection
runs at full clock.
"""

import os

import numpy as np

USE_F32R = os.environ.get("KERNEL_F32R", "0") == "1"

B, T, C = 4, 1024, 1024
NH, HD = 16, 64
NHL = NH // 2          # heads per core (local)
GC = NHL * HD          # channels per head-group = 512
KC = C // 128          # 8 contraction chunks over full C
KCL = GC // 128        # 4 contraction chunks over the local half
SCALE = 1.0 / float(np.sqrt(HD))

_PROG = None


def _build_program():
    import concourse.bacc as bacc
    import concourse.mybir as mybir
    import concourse.tile as tile

    F32 = mybir.dt.float32
    BF16 = mybir.dt.float32r if USE_F32R else mybir.dt.bfloat16
    OBF16 = mybir.dt.bfloat16  # y^T partials: halves the output DMA tail
    Exp = mybir.ActivationFunctionType.Exp
    Ident = mybir.ActivationFunctionType.Identity

    nc = bacc.Bacc()
    xt_d = nc.declare_dram_parameter("xt", [KC, 128, T], BF16, isOutput=False)
    wq_d = nc.declare_dram_parameter("wq", [KCL, 128, C], BF16, isOutput=False)
    wk_d = nc.declare_dram_parameter("wk", [KCL, 128, C], BF16, isOutput=False)
    wv_d = nc.declare_dram_parameter("wv", [KC, 128, GC], BF16, isOutput=False)
    wp_d = nc.declare_dram_parameter("wp", [KCL, 128, C], BF16, isOutput=False)
    bias_d = nc.declare_dram_parameter("bias", [128, 8], F32, isOutput=False)
    yt_d = nc.declare_dram_parameter("yt", [8, 128, T], OBF16, isOutput=True)

    from contextlib import ExitStack

    with ExitStack() as ctx:
        tc = ctx.enter_context(tile.TileContext(nc))
        ctx.enter_context(
            nc.allow_low_precision("bf16 matmul operands are intentional")
        )
        pool = lambda name, bufs, **kw: ctx.enter_context(  # noqa: E731
            tc.tile_pool(name=name, bufs=bufs, **kw)
        )
        xt_pool = pool("xt", KC)
        wqk_pool = pool("wqk", 4)
        wv_pool = pool("wv", KC)
        wp_pool = pool("wp", KCL)
        kt_pool = pool("kt", 2)
        qt_pool = pool("qt", 2)
        v_pool = pool("vaug", KC)
        exp_pool = pool("exp", 18)
        ot_pool = pool("ot", KCL)
        y_pool = pool("ysb", 2)
        r_pool = pool("rsb", 2)
        rbx_pool = pool("rbx", 2)
        ob_pool = pool("osb2", 2)
        misc_pool = pool("misc", 2)
        ps_proj = pool("psproj", 2, space="PSUM")   # [128,512] fp32 = 1 bank
        ps_s = pool("pss", 2, space="PSUM")         # [128,1024] = 2 banks
        ps_ops = pool("psops", 2, space="PSUM")     # [65,512] = 1 bank

        # ---- stage 0: memset ones + streamed loads ------------------
        ones_sb = misc_pool.tile([128, 528], BF16, tag="ones", name="ones_sb")
        nc.gpsimd.memset(ones_sb[:], 1.0)
        bias_t = misc_pool.tile([128, 8], F32, tag="bias", name="bias_t")
        nc.sync.dma_start(bias_t[:], bias_d[:])
        bias_sb = [bias_t[:, m:m + 1] for m in range(8)]

        kw = {0: wqk_pool.tile([128, C], BF16, tag="wqk", name="kw0")}
        nc.sync.dma_start(kw[0][:], wk_d[0])
        xt = []
        for k in range(KC):
            t_ = xt_pool.tile([128, T], BF16, tag="xt", name=f"xt{k}")
            nc.sync.dma_start(t_[:], xt_d[k])
            xt.append(t_)
            if k == 1:
                qw = {0: wqk_pool.tile([128, C], BF16, tag="wqk", name="qw0")}
                nc.sync.dma_start(qw[0][:], wq_d[0])
        wv_sb = []
        for k in range(KC):
            wvt = wv_pool.tile([128, GC], BF16, tag="wv", name=f"wv{k}")
            nc.sync.dma_start(wvt[:], wv_d[k])
            wv_sb.append(wvt)

        def emit_warm(n_warm, wps=None):
            # p-state keep-alive: all-ones matmuls, never read
            if wps is None:
                wps = ps_proj.tile([128, 512], F32, tag="ps", name="warm")
            for _ in range(n_warm):
                nc.tensor.matmul(
                    wps[:], ones_sb[:, 16:144], ones_sb[:, 16:528],
                    start=True, stop=True,
                )
            return wps

        emit_warm(16)

        # Per head the V block is 128 wide: ones in col 0 (PV psum row 0
        # = the denominator Z — the only base the custom-DVE reciprocal
        # and GpSimd broadcast support), v dims in cols 64:128 so the PV
        # output rows sit at the 64-aligned upper half and the
        # normalization multiply can read the psum directly (non-zero
        # partition starts must be 64-aligned) — no SBUF->SBUF bounce.
        va = []
        for m in range(KC):
            vt = v_pool.tile([128, NHL * 128], BF16, tag="vaug", name=f"va{m}")
            view = vt[:].rearrange("p (h e) -> p h e", e=128)
            nc.vector.tensor_copy(
                view[:, :, 0:64],
                ones_sb[:, 0:NHL * 64].rearrange("p (h e) -> p h e", e=64),
            )
            va.append(vt)

        ot = []
        for k in range(KCL):
            o_ = ot_pool.tile([128, T], BF16, tag="ot", name=f"ot{k}")
            ot.append(o_)

        def emit_proj(which, p, wt, dst_pool, warm_between=False):
            d_ = dst_pool.tile([128, T], BF16, tag=which, name=f"{which}{p}")
            wps = emit_warm(0) if warm_between else None
            for n in range(2):
                ps = ps_proj.tile([128, 512], F32, tag="ps", name=f"{which}{p}{n}")
                for k in range(KC):
                    nc.tensor.matmul(
                        ps[:], wt[:, k * 128:(k + 1) * 128],
                        xt[k][:, n * 512:(n + 1) * 512],
                        start=(k == 0), stop=(k == KC - 1),
                    )
                    if warm_between and n == 0 and k < 6:
                        emit_warm(1, wps)
                nc.vector.tensor_copy(d_[:, n * 512:(n + 1) * 512], ps[:])
            return d_

        def emit_s(p, hh, k_, q_):
            # S^T for local head h = 2p+hh over all queries: one
            # [128,1024] psum pair-tile per key chunk j, one exp each.
            po = hh * 64
            exps = []
            for j in range(KC):
                sps = ps_s.tile([128, 2 * 512], F32, tag="ps", name=f"s{p}{hh}{j}")
                for n in range(2):
                    nc.tensor.matmul(
                        sps[:, n * 512:(n + 1) * 512],
                        k_[po:po + 64, j * 128:(j + 1) * 128],
                        q_[po:po + 64, n * 512:(n + 1) * 512],
                        start=True, stop=True,
                    )
                e = exp_pool.tile([128, 2 * 512], BF16, tag="exp",
                                  name=f"e{p}{hh}{j}")
                nc.scalar.activation(e[:], sps[:], Exp, scale=SCALE)
                exps.append(e)
            return exps

        def emit_pv(h, exps):
            # PV for local head h, one chain per query half. Psum row 0
            # is Z, rows 64:128 the head's output dims (64-aligned, so
            # the normalization multiply writes ot directly).
            okc, half = divmod(h, 2)
            po = half * 64
            for n in range(2):
                ops = ps_ops.tile([128, 512], F32, tag="ps", name=f"o{h}{n}")
                for j in range(KC):
                    nc.tensor.matmul(
                        ops[:], va[j][:, h * 128:(h + 1) * 128],
                        exps[j][:, n * 512:(n + 1) * 512],
                        start=(j == 0), stop=(j == KC - 1),
                    )
                rt0 = r_pool.tile([1, 512], F32, tag="rsb", name=f"r{h}{n}")
                nc.vector.reciprocal_approx_fast(rt0[0:1, :], ops[0:1, :])
                rbx = rbx_pool.tile([64, 512], F32, tag="rbx", name=f"rbx{h}{n}")
                nc.gpsimd.partition_broadcast(rbx[:], rt0[0:1, :])
                nc.vector.tensor_mul(
                    ot[okc][po:po + 64, n * 512:(n + 1) * 512],
                    ops[64:128, :], rbx[0:64, :],
                )

        # ---- pair 0 head: K/Q/S(h0), V stage, S(h1) -----------------
        kt_ = emit_proj("kt", 0, kw[0], kt_pool, warm_between=True)
        qt_ = emit_proj("qt", 0, qw[0], qt_pool)
        exps_a = emit_s(0, 0, kt_, qt_)

        # V = x @ Wv^T ([tok, chan], ones col 0); ACT drains EXPs(h0)
        for m in range(KC):
            ps = ps_proj.tile([128, GC], F32, tag="ps", name=f"v{m}")
            for k in range(KC):
                nc.tensor.matmul(
                    ps[:], xt[k][:, m * 128:(m + 1) * 128], wv_sb[k][:],
                    start=(k == 0), stop=(k == KC - 1),
                )
            view = va[m][:].rearrange("p (h e) -> p h e", e=128)
            nc.vector.tensor_copy(
                view[:, :, 64:128], ps[:].rearrange("p (h d) -> p h d", d=64)
            )

        kw[1] = wqk_pool.tile([128, C], BF16, tag="wqk", name="kw1")
        nc.sync.dma_start(kw[1][:], wk_d[1])
        qw[1] = wqk_pool.tile([128, C], BF16, tag="wqk", name="qw1")
        nc.sync.dma_start(qw[1][:], wq_d[1])

        exps_b = emit_s(0, 1, kt_, qt_)
        emit_pv(0, exps_a)
        emit_pv(1, exps_b)

        # ---- pairs 1..3 ---------------------------------------------
        yw_sb = {}
        y_ps = {}

        def y_partial(m):
            ps = ps_s.tile([128, 2 * 512], F32, tag="ps", name=f"y{m}")
            for kc in (0, 1, 2):
                for n in range(2):
                    nc.tensor.matmul(
                        ps[:, n * 512:(n + 1) * 512],
                        yw_sb[kc][:, m * 128:(m + 1) * 128],
                        ot[kc][:, n * 512:(n + 1) * 512],
                        start=(kc == 0), stop=False,
                    )
            y_ps[m] = ps

        def y_finish(m):
            ps = y_ps.pop(m, None)
            if ps is None:
                ps = ps_s.tile([128, 2 * 512], F32, tag="ps", name=f"y{m}")
                kcs = (0, 1, 2, 3)
            else:
                kcs = (3,)
            for kc in kcs:
                for n in range(2):
                    nc.tensor.matmul(
                        ps[:, n * 512:(n + 1) * 512],
                        yw_sb[kc][:, m * 128:(m + 1) * 128],
                        ot[kc][:, n * 512:(n + 1) * 512],
                        start=(kc == 0), stop=(kc == KCL - 1),
                    )
            ysb = y_pool.tile([128, T], OBF16, tag="ysb", name=f"ysb{m}")
            nc.scalar.activation(ysb[:], ps[:], Ident, bias=bias_sb[m])
            nc.sync.dma_start(yt_d[m], ysb[:])

        for p in range(1, KCL):
            kt_ = emit_proj("kt", p, kw[p], kt_pool)
            qt_ = emit_proj("qt", p, qw[p], qt_pool)
            if p + 1 < KCL:
                kw[p + 1] = wqk_pool.tile([128, C], BF16, tag="wqk",
                                          name=f"kw{p + 1}")
                nc.sync.dma_start(kw[p + 1][:], wk_d[p + 1])
                qw[p + 1] = wqk_pool.tile([128, C], BF16, tag="wqk",
                                          name=f"qw{p + 1}")
                nc.sync.dma_start(qw[p + 1][:], wq_d[p + 1])
            for m in (2 * (p - 1), 2 * (p - 1) + 1):
                if m < KCL:
                    yw = wp_pool.tile([128, C], BF16, tag="wp", name=f"yw{m}")
                    nc.sync.dma_start(yw[:], wp_d[m])
                    yw_sb[m] = yw
            # S of both heads first: PV(a)'s EXP backlog drains behind
            # S(b)'s matmuls instead of stalling the in-order PE queue.
            exps_a = emit_s(p, 0, kt_, qt_)
            exps_b = emit_s(p, 1, kt_, qt_)
            emit_pv(2 * p, exps_a)
            emit_pv(2 * p + 1, exps_b)

        # ---- output projection --------------------------------------
        y_partial(0)
        y_partial(1)
        y_finish(0)
        y_finish(1)
        for m in range(2, 8):
            y_finish(m)

    nc.compile()
    return nc


def _get_program():
    global _PROG
    if _PROG is None:
        _PROG = _build_program()
    return _PROG


def _prep_inputs(x, Wqkv, Wproj, bproj):
    """Host-side shard prep: per-core input maps."""
    import ml_dtypes

    bf16 = np.float32 if USE_F32R else ml_dtypes.bfloat16
    x = np.asarray(x, dtype=np.float32)
    Wqkv = np.asarray(Wqkv, dtype=np.float32)
    Wproj = np.asarray(Wproj, dtype=np.float32)
    bproj = np.asarray(bproj, dtype=np.float32)

    def stat_cols(wT_slice):
        # [C, GC] (in chan, local out) -> [KCL, 128, C] stationary tiles:
        # tile[p][part, kc*128+m] = wT_slice[kc*128+part, p*128+m]
        return np.ascontiguousarray(
            wT_slice.reshape(KC, 128, KCL, 128)
            .transpose(2, 1, 0, 3)
            .reshape(KCL, 128, C)
        ).astype(bf16)

    bias = np.ascontiguousarray(bproj.reshape(8, 128).T)
    zbias = np.zeros_like(bias)

    per_g = []
    for g in range(2):
        lo = GC * g
        wq = stat_cols(Wqkv[0 * C + lo:0 * C + lo + GC].T)
        wk = stat_cols(Wqkv[1 * C + lo:1 * C + lo + GC].T)
        wv = np.ascontiguousarray(
            Wqkv[2 * C + lo:2 * C + lo + GC].T.reshape(KC, 128, GC)
        ).astype(bf16)
        # yw[kc][part, m*128+d] = Wproj^T[lo + kc*128 + part, m*128+d]
        wp = np.ascontiguousarray(
            Wproj.T[lo:lo + GC].reshape(KCL, 128, C)
        ).astype(bf16)
        per_g.append((wq, wk, wv, wp))

    in_maps = []
    for i in range(8):
        b, g = divmod(i, 2)
        wq, wk, wv, wp = per_g[g]
        xt = np.ascontiguousarray(x[b].T.reshape(KC, 128, T)).astype(bf16)
        in_maps.append(
            {
                "xt": xt, "wq": wq, "wk": wk, "wv": wv, "wp": wp,
                "bias": bias if g == 0 else zbias,
            }
        )
    return in_maps


def _assemble(results, x_dtype):
    out = np.empty((B, T, C), dtype=np.float32)
    for b in range(B):
        yt = results[2 * b]["yt"].reshape(C, T).astype(np.float32)
        yt = yt + results[2 * b + 1]["yt"].reshape(C, T)
        out[b] = yt.T
    return out.astype(x_dtype, copy=False)


def run(inputs, trace=False, **spmd_kwargs):
    """Shared entry for kernel() and test harnesses (trace for profiling)."""
    from concourse.bass_utils import run_bass_kernel_spmd

    nc = _get_program()
    in_maps = _prep_inputs(**inputs)
    res = run_bass_kernel_spmd(
        nc, in_maps, list(range(8)), trace=trace, **spmd_kwargs
    )
    out = _assemble(res.results, np.asarray(inputs["x"]).dtype)
    return out, res


def kernel(x, Wqkv, Wproj, bproj):
    out, _ = run(dict(x=x, Wqkv=Wqkv, Wproj=Wproj, bproj=bproj))
    return out


# revision 37
# speedup vs baseline: 1.2651x; 1.2651x over previous
"""Trainium2 Bass kernel for nn_Attention (B=4, T=1024, C=1024, 16 heads).

Sharding: 8 cores = (batch b, head-group g). Core i = 2b+g handles heads
[8g, 8g+8) of batch b for ALL 1024 tokens — tensor-parallel over heads,
column-parallel Wproj. Zero redundant FLOPs (512 matmuls/core vs 648 for
the old batch×query-half scheme, which recomputed K/V for the whole
batch on both cores of a pair). Each core emits the partial
y^T = Wproj[:, cols g] @ out^T_g for all tokens; the host unshard sums
the two partials per batch (a 4M-element add, same order as the
transpose it already does).

Everything on-chip is laid out so no transposes are ever needed:
  - the host passes x^T; Q^T/K^T come out of their projections in
    [chan, tok] layout directly
  - V is produced in [tok, chan] layout with a leading ones-column per
    head, so the PV matmul's PSUM row 0 is the softmax denominator Z
    (row 0 is the only base the custom-DVE reciprocal and GpSimd
    partition_broadcast support)
  - softmax runs on S^T (keys on partitions) with no max-subtraction
    (logits are O(6), exp is safe); one [128,1024] exp covers both
    query halves of a head
  - normalization: fast-DVE reciprocal of the Z row, partition
    broadcast, one DVE multiply, and a SBUF->SBUF DMA shifts rows 1:65
    into the out^T tile

All matmul operands are bf16 (same 1 cycle/row PE rate as float32r but
half the HBM/DMA traffic; an fp32r A/B measured 238us vs 174us for the
same structure). PSUM accumulation is fp32. Rel err vs the fp32
reference ~8e-3 (gate 2e-2).

Schedule: the ScalarE engine is ~50% busy with 64 [128,1024] EXPs
(1.09us each), and the in-order PE queue would stall on them, so every
EXP-gated PV chain is preceded by ~3.6us of EXP-independent matmul
work: S of both heads first, then the next pair's K^T projection
between, then PV of head a, then the next pair's Q^T projection, then
PV of head b. Warm-up matmuls on a memset ones tile ramp the PE
p-state during the initial x^T DMA (the PE drops to 1.2/0.65 GHz after
any >~1us idle gap and takes ~3us of continuous work to re-reach
2.4 GHz); pair 3 uses warm fills and the first Y tiles' kc0..2 partial
contractions to bridge the final norm chains, so the output projection
runs at full clock.
"""

import os

import numpy as np

USE_F32R = os.environ.get("KERNEL_F32R", "0") == "1"

B, T, C = 4, 1024, 1024
NH, HD = 16, 64
NHL = NH // 2          # heads per core (local)
GC = NHL * HD          # channels per head-group = 512
KC = C // 128          # 8 contraction chunks over full C
KCL = GC // 128        # 4 contraction chunks over the local half
SCALE = 1.0 / float(np.sqrt(HD))

_PROG = None


def _build_program():
    import concourse.bacc as bacc
    import concourse.mybir as mybir
    import concourse.tile as tile

    F32 = mybir.dt.float32
    BF16 = mybir.dt.float32r if USE_F32R else mybir.dt.bfloat16
    OBF16 = mybir.dt.bfloat16  # y^T partials: halves the output DMA tail
    Exp = mybir.ActivationFunctionType.Exp
    Ident = mybir.ActivationFunctionType.Identity

    nc = bacc.Bacc()
    xt_d = nc.declare_dram_parameter("xt", [KC, 128, T], BF16, isOutput=False)
    wq_d = nc.declare_dram_parameter("wq", [KCL, 128, C], BF16, isOutput=False)
    wk_d = nc.declare_dram_parameter("wk", [KCL, 128, C], BF16, isOutput=False)
    wv_d = nc.declare_dram_parameter("wv", [KC, 128, GC], BF16, isOutput=False)
    wp_d = nc.declare_dram_parameter("wp", [KCL, 128, C], BF16, isOutput=False)
    bias_d = nc.declare_dram_parameter("bias", [128, 8], F32, isOutput=False)
    yt_d = nc.declare_dram_parameter("yt", [8, 128, T], OBF16, isOutput=True)

    from contextlib import ExitStack

    with ExitStack() as ctx:
        tc = ctx.enter_context(tile.TileContext(nc))
        ctx.enter_context(
            nc.allow_low_precision("bf16 matmul operands are intentional")
        )
        pool = lambda name, bufs, **kw: ctx.enter_context(  # noqa: E731
            tc.tile_pool(name=name, bufs=bufs, **kw)
        )
        xt_pool = pool("xt", KC)
        wqk_pool = pool("wqk", 4)
        wv_pool = pool("wv", KC)
        wp_pool = pool("wp", KCL)
        kt_pool = pool("kt", 2)
        qt_pool = pool("qt", 2)
        v_pool = pool("vaug", KC)
        exp_pool = pool("exp", 18)
        ot_pool = pool("ot", KCL)
        y_pool = pool("ysb", 2)
        r_pool = pool("rsb", 2)
        rbx_pool = pool("rbx", 2)
        misc_pool = pool("misc", 2)
        ps_proj = pool("psproj", 2, space="PSUM")   # [128,512] fp32 = 1 bank
        ps_s = pool("pss", 2, space="PSUM")         # [128,1024] = 2 banks
        ps_ops = pool("psops", 2, space="PSUM")     # [65,512] = 1 bank

        # ---- stage 0: memset ones + streamed loads ------------------
        ones_sb = misc_pool.tile([128, 528], BF16, tag="ones", name="ones_sb")
        nc.gpsimd.memset(ones_sb[:], 1.0)
        bias_t = misc_pool.tile([128, 8], F32, tag="bias", name="bias_t")
        nc.sync.dma_start(bias_t[:], bias_d[:])
        bias_sb = [bias_t[:, m:m + 1] for m in range(8)]

        kw = {0: wqk_pool.tile([128, C], BF16, tag="wqk", name="kw0")}
        nc.sync.dma_start(kw[0][:], wk_d[0])
        xt = []
        for k in range(KC):
            t_ = xt_pool.tile([128, T], BF16, tag="xt", name=f"xt{k}")
            nc.sync.dma_start(t_[:], xt_d[k])
            xt.append(t_)
            if k == 1:
                qw = {0: wqk_pool.tile([128, C], BF16, tag="wqk", name="qw0")}
                nc.sync.dma_start(qw[0][:], wq_d[0])
        wv_sb = []
        for k in range(KC):
            wvt = wv_pool.tile([128, GC], BF16, tag="wv", name=f"wv{k}")
            nc.sync.dma_start(wvt[:], wv_d[k])
            wv_sb.append(wvt)

        def emit_warm(n_warm, wps=None):
            # p-state keep-alive: all-ones matmuls, never read
            if wps is None:
                wps = ps_proj.tile([128, 512], F32, tag="ps", name="warm")
            for _ in range(n_warm):
                nc.tensor.matmul(
                    wps[:], ones_sb[:, 16:144], ones_sb[:, 16:528],
                    start=True, stop=True,
                )
            return wps

        emit_warm(16)

        # Per head the V block is 128 wide: ones in col 0 (PV psum row 0
        # = the denominator Z — the only base the custom-DVE reciprocal
        # and GpSimd broadcast support), v dims in cols 64:128 so the PV
        # output rows sit at the 64-aligned upper half and the
        # normalization multiply can read the psum directly (non-zero
        # partition starts must be 64-aligned) — no SBUF->SBUF bounce.
        va = []
        for m in range(KC):
            vt = v_pool.tile([128, NHL * 128], BF16, tag="vaug", name=f"va{m}")
            view = vt[:].rearrange("p (h e) -> p h e", e=128)
            nc.vector.tensor_copy(
                view[:, :, 0:64],
                ones_sb[:, 0:NHL * 64].rearrange("p (h e) -> p h e", e=64),
            )
            va.append(vt)

        ot = []
        for k in range(KCL):
            o_ = ot_pool.tile([128, T], BF16, tag="ot", name=f"ot{k}")
            ot.append(o_)

        def emit_proj(which, p, wt, dst_pool, warm_between=False):
            d_ = dst_pool.tile([128, T], BF16, tag=which, name=f"{which}{p}")
            wps = emit_warm(0) if warm_between else None
            for n in range(2):
                ps = ps_proj.tile([128, 512], F32, tag="ps", name=f"{which}{p}{n}")
                for k in range(KC):
                    nc.tensor.matmul(
                        ps[:], wt[:, k * 128:(k + 1) * 128],
                        xt[k][:, n * 512:(n + 1) * 512],
                        start=(k == 0), stop=(k == KC - 1),
                    )
                    if warm_between and n == 0 and k < 6:
                        emit_warm(1, wps)
                nc.vector.tensor_copy(d_[:, n * 512:(n + 1) * 512], ps[:])
            return d_

        def emit_s(p, hh, k_, q_):
            # S^T for local head h = 2p+hh over all queries: one
            # [128,1024] psum pair-tile per key chunk j, one exp each.
            po = hh * 64
            exps = []
            for j in range(KC):
                sps = ps_s.tile([128, 2 * 512], F32, tag="ps", name=f"s{p}{hh}{j}")
                for n in range(2):
                    nc.tensor.matmul(
                        sps[:, n * 512:(n + 1) * 512],
                        k_[po:po + 64, j * 128:(j + 1) * 128],
                        q_[po:po + 64, n * 512:(n + 1) * 512],
                        start=True, stop=True,
                    )
                e = exp_pool.tile([128, 2 * 512], BF16, tag="exp",
                                  name=f"e{p}{hh}{j}")
                nc.scalar.activation(e[:], sps[:], Exp, scale=SCALE)
                exps.append(e)
            return exps

        def emit_pv(h, n, exps):
            # PV for local head h, query half n. Psum row 0 is Z, rows
            # 64:128 the head's output dims (64-aligned, so the
            # normalization multiply writes ot directly).
            ops = ps_ops.tile([128, 512], F32, tag="ps", name=f"o{h}{n}")
            for j in range(KC):
                nc.tensor.matmul(
                    ops[:], va[j][:, h * 128:(h + 1) * 128],
                    exps[j][:, n * 512:(n + 1) * 512],
                    start=(j == 0), stop=(j == KC - 1),
                )
            okc, half = divmod(h, 2)
            po = half * 64
            rt0 = r_pool.tile([1, 512], F32, tag="rsb", name=f"r{h}{n}")
            nc.vector.reciprocal_approx_fast(rt0[0:1, :], ops[0:1, :])
            rbx = rbx_pool.tile([64, 512], F32, tag="rbx", name=f"rbx{h}{n}")
            nc.gpsimd.partition_broadcast(rbx[:], rt0[0:1, :])
            nc.vector.tensor_mul(
                ot[okc][po:po + 64, n * 512:(n + 1) * 512],
                ops[64:128, :], rbx[0:64, :],
            )

        # ---- pair 0 head: K/Q/S(h0), V stage, S(h1) -----------------
        kt_ = emit_proj("kt", 0, kw[0], kt_pool, warm_between=True)
        qt_ = emit_proj("qt", 0, qw[0], qt_pool)
        exps_a = emit_s(0, 0, kt_, qt_)

        # V = x @ Wv^T ([tok, chan], ones col 0); ACT drains EXPs(h0)
        for m in range(KC):
            ps = ps_proj.tile([128, GC], F32, tag="ps", name=f"v{m}")
            for k in range(KC):
                nc.tensor.matmul(
                    ps[:], xt[k][:, m * 128:(m + 1) * 128], wv_sb[k][:],
                    start=(k == 0), stop=(k == KC - 1),
                )
            view = va[m][:].rearrange("p (h e) -> p h e", e=128)
            nc.vector.tensor_copy(
                view[:, :, 64:128], ps[:].rearrange("p (h d) -> p h d", d=64)
            )

        kw[1] = wqk_pool.tile([128, C], BF16, tag="wqk", name="kw1")
        nc.sync.dma_start(kw[1][:], wk_d[1])
        qw[1] = wqk_pool.tile([128, C], BF16, tag="wqk", name="qw1")
        nc.sync.dma_start(qw[1][:], wq_d[1])

        exps_b = emit_s(0, 1, kt_, qt_)

        # ---- main pair loop -----------------------------------------
        # Between EXP-gated PV chains, keep the PE fed with the next
        # pair's projections (pairs 0-2) or warms + Y partials (pair 3).
        yw_sb = {}
        y_ps = {}

        def y_partial(m):
            ps = ps_s.tile([128, 2 * 512], F32, tag="ps", name=f"y{m}")
            for kc in (0, 1, 2):
                for n in range(2):
                    nc.tensor.matmul(
                        ps[:, n * 512:(n + 1) * 512],
                        yw_sb[kc][:, m * 128:(m + 1) * 128],
                        ot[kc][:, n * 512:(n + 1) * 512],
                        start=(kc == 0), stop=False,
                    )
            y_ps[m] = ps

        def y_finish(m):
            ps = y_ps.pop(m, None)
            if ps is None:
                ps = ps_s.tile([128, 2 * 512], F32, tag="ps", name=f"y{m}")
                kcs = (0, 1, 2, 3)
            else:
                kcs = (3,)
            for kc in kcs:
                for n in range(2):
                    nc.tensor.matmul(
                        ps[:, n * 512:(n + 1) * 512],
                        yw_sb[kc][:, m * 128:(m + 1) * 128],
                        ot[kc][:, n * 512:(n + 1) * 512],
                        start=(kc == 0), stop=(kc == KCL - 1),
                    )
            ysb = y_pool.tile([128, T], OBF16, tag="ysb", name=f"ysb{m}")
            nc.scalar.activation(ysb[:], ps[:], Ident, bias=bias_sb[m])
            nc.sync.dma_start(yt_d[m], ysb[:])

        for p in range(KCL):
            a, b = 2 * p, 2 * p + 1
            if p > 0:
                exps_a = emit_s(p, 0, kt_, qt_)
                exps_b = emit_s(p, 1, kt_, qt_)
            # filler 1: next pair's K^T (or warms for the last pair)
            if p < KCL - 1:
                ktn = emit_proj("kt", p + 1, kw[p + 1], kt_pool)
            else:
                emit_warm(8)
            for n in range(2):
                emit_pv(a, n, exps_a)
            # filler 2: next pair's Q^T (or Y partials for the last pair)
            if p < KCL - 1:
                qtn = emit_proj("qt", p + 1, qw[p + 1], qt_pool)
            else:
                y_partial(0)
                y_partial(1)
            for n in range(2):
                emit_pv(b, n, exps_b)
            if p < KCL - 1:
                kt_, qt_ = ktn, qtn
                # prefetch weights for pair p+2 and the Y stage
                if p + 2 < KCL:
                    kw[p + 2] = wqk_pool.tile([128, C], BF16, tag="wqk",
                                              name=f"kw{p + 2}")
                    nc.sync.dma_start(kw[p + 2][:], wk_d[p + 2])
                    qw[p + 2] = wqk_pool.tile([128, C], BF16, tag="wqk",
                                              name=f"qw{p + 2}")
                    nc.sync.dma_start(qw[p + 2][:], wq_d[p + 2])
                for m in (2 * p, 2 * p + 1):
                    if m < KCL:
                        yw = wp_pool.tile([128, C], BF16, tag="wp",
                                          name=f"yw{m}")
                        nc.sync.dma_start(yw[:], wp_d[m])
                        yw_sb[m] = yw

        # ---- output projection --------------------------------------
        emit_warm(4)
        for m in range(8):
            y_finish(m)

    nc.compile()
    return nc


def _get_program():
    global _PROG
    if _PROG is None:
        _PROG = _build_program()
    return _PROG


def _prep_inputs(x, Wqkv, Wproj, bproj):
    """Host-side shard prep: per-core input maps."""
    import ml_dtypes

    bf16 = np.float32 if USE_F32R else ml_dtypes.bfloat16
    x = np.asarray(x, dtype=np.float32)
    Wqkv = np.asarray(Wqkv, dtype=np.float32)
    Wproj = np.asarray(Wproj, dtype=np.float32)
    bproj = np.asarray(bproj, dtype=np.float32)

    def stat_cols(wT_slice):
        # [C, GC] (in chan, local out) -> [KCL, 128, C] stationary tiles:
        # tile[p][part, kc*128+m] = wT_slice[kc*128+part, p*128+m]
        return np.ascontiguousarray(
            wT_slice.reshape(KC, 128, KCL, 128)
            .transpose(2, 1, 0, 3)
            .reshape(KCL, 128, C)
        ).astype(bf16)

    bias = np.ascontiguousarray(bproj.reshape(8, 128).T)
    zbias = np.zeros_like(bias)

    per_g = []
    for g in range(2):
        lo = GC * g
        wq = stat_cols(Wqkv[0 * C + lo:0 * C + lo + GC].T)
        wk = stat_cols(Wqkv[1 * C + lo:1 * C + lo + GC].T)
        wv = np.ascontiguousarray(
            Wqkv[2 * C + lo:2 * C + lo + GC].T.reshape(KC, 128, GC)
        ).astype(bf16)
        # yw[kc][part, m*128+d] = Wproj^T[lo + kc*128 + part, m*128+d]
        wp = np.ascontiguousarray(
            Wproj.T[lo:lo + GC].reshape(KCL, 128, C)
        ).astype(bf16)
        per_g.append((wq, wk, wv, wp))

    in_maps = []
    for i in range(8):
        b, g = divmod(i, 2)
        wq, wk, wv, wp = per_g[g]
        xt = np.ascontiguousarray(x[b].T.reshape(KC, 128, T)).astype(bf16)
        in_maps.append(
            {
                "xt": xt, "wq": wq, "wk": wk, "wv": wv, "wp": wp,
                "bias": bias if g == 0 else zbias,
            }
        )
    return in_maps


def _assemble(results, x_dtype):
    out = np.empty((B, T, C), dtype=np.float32)
    for b in range(B):
        yt = results[2 * b]["yt"].reshape(C, T).astype(np.float32)
        yt = yt + results[2 * b + 1]["yt"].reshape(C, T)
        out[b] = yt.T
    return out.astype(x_dtype, copy=False)


def run(inputs, trace=False, **spmd_kwargs):
    """Shared entry for kernel() and test harnesses (trace for profiling)."""
    from concourse.bass_utils import run_bass_kernel_spmd

    nc = _get_program()
    in_maps = _prep_inputs(**inputs)
    res = run_bass_kernel_spmd(
        nc, in_maps, list(range(8)), trace=trace, **spmd_kwargs
    )
    out = _assemble(res.results, np.asarray(inputs["x"]).dtype)
    return out, res


def kernel(x, Wqkv, Wproj, bproj):
    out, _ = run(dict(x=x, Wqkv=Wqkv, Wproj=Wproj, bproj=bproj))
    return out


# revision 38
# speedup vs baseline: 1.2882x; 1.0182x over previous
"""Trainium2 Bass kernel for nn_Attention (B=4, T=1024, C=1024, 16 heads).

Sharding: 8 cores = (batch b, head-group g). Core i = 2b+g handles heads
[8g, 8g+8) of batch b for ALL 1024 tokens — tensor-parallel over heads,
column-parallel Wproj. Zero redundant FLOPs (512 matmuls/core vs 648 for
the old batch×query-half scheme, which recomputed K/V for the whole
batch on both cores of a pair). Each core emits the partial
y^T = Wproj[:, cols g] @ out^T_g for all tokens; the host unshard sums
the two partials per batch (a 4M-element add, same order as the
transpose it already does).

Everything on-chip is laid out so no transposes are ever needed:
  - the host passes x^T; Q^T/K^T come out of their projections in
    [chan, tok] layout directly
  - V is produced in [tok, chan] layout with a leading ones-column per
    head, so the PV matmul's PSUM row 0 is the softmax denominator Z
    (row 0 is the only base the custom-DVE reciprocal and GpSimd
    partition_broadcast support)
  - softmax runs on S^T (keys on partitions) with no max-subtraction
    (logits are O(6), exp is safe); one [128,1024] exp covers both
    query halves of a head
  - normalization: fast-DVE reciprocal of the Z row, partition
    broadcast, one DVE multiply, and a SBUF->SBUF DMA shifts rows 1:65
    into the out^T tile

All matmul operands are bf16 (same 1 cycle/row PE rate as float32r but
half the HBM/DMA traffic; an fp32r A/B measured 238us vs 174us for the
same structure). PSUM accumulation is fp32. Rel err vs the fp32
reference ~8e-3 (gate 2e-2).

Schedule: the ScalarE engine is ~50% busy with 64 [128,1024] EXPs
(1.09us each), and the in-order PE queue would stall on them, so every
EXP-gated PV chain is preceded by ~3.6us of EXP-independent matmul
work: S of both heads first, then the next pair's K^T projection
between, then PV of head a, then the next pair's Q^T projection, then
PV of head b. Warm-up matmuls on a memset ones tile ramp the PE
p-state during the initial x^T DMA (the PE drops to 1.2/0.65 GHz after
any >~1us idle gap and takes ~3us of continuous work to re-reach
2.4 GHz); pair 3 uses warm fills and the first Y tiles' kc0..2 partial
contractions to bridge the final norm chains, so the output projection
runs at full clock.
"""

import os

import numpy as np

USE_F32R = os.environ.get("KERNEL_F32R", "0") == "1"

B, T, C = 4, 1024, 1024
NH, HD = 16, 64
NHL = NH // 2          # heads per core (local)
GC = NHL * HD          # channels per head-group = 512
KC = C // 128          # 8 contraction chunks over full C
KCL = GC // 128        # 4 contraction chunks over the local half
SCALE = 1.0 / float(np.sqrt(HD))

_PROG = None


def _build_program():
    import concourse.bacc as bacc
    import concourse.mybir as mybir
    import concourse.tile as tile

    F32 = mybir.dt.float32
    BF16 = mybir.dt.float32r if USE_F32R else mybir.dt.bfloat16
    OBF16 = mybir.dt.bfloat16  # y^T partials: halves the output DMA tail
    Exp = mybir.ActivationFunctionType.Exp
    Ident = mybir.ActivationFunctionType.Identity

    nc = bacc.Bacc()
    xt_d = nc.declare_dram_parameter("xt", [KC, 128, T], BF16, isOutput=False)
    wq_d = nc.declare_dram_parameter("wq", [KCL, 128, C], BF16, isOutput=False)
    wk_d = nc.declare_dram_parameter("wk", [KCL, 128, C], BF16, isOutput=False)
    wv_d = nc.declare_dram_parameter("wv", [KC, 128, GC], BF16, isOutput=False)
    wp_d = nc.declare_dram_parameter("wp", [KCL, 128, C], BF16, isOutput=False)
    bias_d = nc.declare_dram_parameter("bias", [128, 8], F32, isOutput=False)
    yt_d = nc.declare_dram_parameter("yt", [8, 128, T], OBF16, isOutput=True)

    from contextlib import ExitStack

    with ExitStack() as ctx:
        tc = ctx.enter_context(tile.TileContext(nc))
        ctx.enter_context(
            nc.allow_low_precision("bf16 matmul operands are intentional")
        )
        pool = lambda name, bufs, **kw: ctx.enter_context(  # noqa: E731
            tc.tile_pool(name=name, bufs=bufs, **kw)
        )
        xt_pool = pool("xt", KC)
        wqk_pool = pool("wqk", 4)
        wv_pool = pool("wv", KC)
        wp_pool = pool("wp", KCL)
        kt_pool = pool("kt", 2)
        qt_pool = pool("qt", 2)
        v_pool = pool("vaug", KC)
        exp_pool = pool("exp", 18)
        ot_pool = pool("ot", KCL)
        y_pool = pool("ysb", 2)
        r_pool = pool("rsb", 2)
        rbx_pool = pool("rbx", 2)
        misc_pool = pool("misc", 2)
        ps_proj = pool("psproj", 2, space="PSUM")   # [128,512] fp32 = 1 bank
        ps_s = pool("pss", 2, space="PSUM")         # [128,1024] = 2 banks
        ps_ops = pool("psops", 2, space="PSUM")     # [65,512] = 1 bank

        # ---- stage 0: memset ones + streamed loads ------------------
        ones_sb = misc_pool.tile([128, 528], BF16, tag="ones", name="ones_sb")
        nc.gpsimd.memset(ones_sb[:], 1.0)
        bias_t = misc_pool.tile([128, 8], F32, tag="bias", name="bias_t")
        nc.sync.dma_start(bias_t[:], bias_d[:])
        bias_sb = [bias_t[:, m:m + 1] for m in range(8)]

        # x^T loads split by column half, all first halves ahead of the
        # second: the kt0/qt0 n=0 psum chains only read columns 0:512,
        # so they start ~2x earlier than with whole-tile loads.
        kw = {0: wqk_pool.tile([128, C], BF16, tag="wqk", name="kw0")}
        nc.sync.dma_start(kw[0][:], wk_d[0])
        xt = []
        for k in range(KC):
            t_ = xt_pool.tile([128, T], BF16, tag="xt", name=f"xt{k}")
            nc.sync.dma_start(t_[:, 0:512], xt_d[k][:, 0:512])
            xt.append(t_)
            if k == 1:
                qw = {0: wqk_pool.tile([128, C], BF16, tag="wqk", name="qw0")}
                nc.sync.dma_start(qw[0][:], wq_d[0])
        for k in range(KC):
            nc.sync.dma_start(xt[k][:, 512:T], xt_d[k][:, 512:T])
        wv_sb = []
        for k in range(KC):
            wvt = wv_pool.tile([128, GC], BF16, tag="wv", name=f"wv{k}")
            nc.sync.dma_start(wvt[:], wv_d[k])
            wv_sb.append(wvt)

        def emit_warm(n_warm, wps=None):
            # p-state keep-alive: all-ones matmuls, never read
            if wps is None:
                wps = ps_proj.tile([128, 512], F32, tag="ps", name="warm")
            for _ in range(n_warm):
                nc.tensor.matmul(
                    wps[:], ones_sb[:, 16:144], ones_sb[:, 16:528],
                    start=True, stop=True,
                )
            return wps

        emit_warm(16)

        # Per head the V block is 128 wide: ones in col 0 (PV psum row 0
        # = the denominator Z — the only base the custom-DVE reciprocal
        # and GpSimd broadcast support), v dims in cols 64:128 so the PV
        # output rows sit at the 64-aligned upper half and the
        # normalization multiply can read the psum directly (non-zero
        # partition starts must be 64-aligned) — no SBUF->SBUF bounce.
        va = []
        for m in range(KC):
            vt = v_pool.tile([128, NHL * 128], BF16, tag="vaug", name=f"va{m}")
            view = vt[:].rearrange("p (h e) -> p h e", e=128)
            nc.vector.tensor_copy(
                view[:, :, 0:64],
                ones_sb[:, 0:NHL * 64].rearrange("p (h e) -> p h e", e=64),
            )
            va.append(vt)

        ot = []
        for k in range(KCL):
            o_ = ot_pool.tile([128, T], BF16, tag="ot", name=f"ot{k}")
            ot.append(o_)

        def emit_proj(which, p, wt, dst_pool, warm_between=False):
            d_ = dst_pool.tile([128, T], BF16, tag=which, name=f"{which}{p}")
            wps = emit_warm(0) if warm_between else None
            for n in range(2):
                ps = ps_proj.tile([128, 512], F32, tag="ps", name=f"{which}{p}{n}")
                for k in range(KC):
                    nc.tensor.matmul(
                        ps[:], wt[:, k * 128:(k + 1) * 128],
                        xt[k][:, n * 512:(n + 1) * 512],
                        start=(k == 0), stop=(k == KC - 1),
                    )
                    if warm_between and n == 0 and k < 6:
                        emit_warm(1, wps)
                nc.vector.tensor_copy(d_[:, n * 512:(n + 1) * 512], ps[:])
            return d_

        def emit_s(p, hh, k_, q_):
            # S^T for local head h = 2p+hh over all queries: one
            # [128,1024] psum pair-tile per key chunk j, one exp each.
            po = hh * 64
            exps = []
            for j in range(KC):
                sps = ps_s.tile([128, 2 * 512], F32, tag="ps", name=f"s{p}{hh}{j}")
                for n in range(2):
                    nc.tensor.matmul(
                        sps[:, n * 512:(n + 1) * 512],
                        k_[po:po + 64, j * 128:(j + 1) * 128],
                        q_[po:po + 64, n * 512:(n + 1) * 512],
                        start=True, stop=True,
                    )
                e = exp_pool.tile([128, 2 * 512], BF16, tag="exp",
                                  name=f"e{p}{hh}{j}")
                nc.scalar.activation(e[:], sps[:], Exp, scale=SCALE)
                exps.append(e)
            return exps

        def emit_pv(h, n, exps):
            # PV for local head h, query half n. Psum row 0 is Z, rows
            # 64:128 the head's output dims (64-aligned, so the
            # normalization multiply writes ot directly).
            ops = ps_ops.tile([128, 512], F32, tag="ps", name=f"o{h}{n}")
            for j in range(KC):
                nc.tensor.matmul(
                    ops[:], va[j][:, h * 128:(h + 1) * 128],
                    exps[j][:, n * 512:(n + 1) * 512],
                    start=(j == 0), stop=(j == KC - 1),
                )
            okc, half = divmod(h, 2)
            po = half * 64
            rt0 = r_pool.tile([1, 512], F32, tag="rsb", name=f"r{h}{n}")
            nc.vector.reciprocal_approx_fast(rt0[0:1, :], ops[0:1, :])
            rbx = rbx_pool.tile([64, 512], F32, tag="rbx", name=f"rbx{h}{n}")
            nc.gpsimd.partition_broadcast(rbx[:], rt0[0:1, :])
            nc.vector.tensor_mul(
                ot[okc][po:po + 64, n * 512:(n + 1) * 512],
                ops[64:128, :], rbx[0:64, :],
            )

        # ---- pair 0 head: K/Q/S(h0), V stage, S(h1) -----------------
        kt_ = emit_proj("kt", 0, kw[0], kt_pool, warm_between=True)
        qt_ = emit_proj("qt", 0, qw[0], qt_pool)
        exps_a = emit_s(0, 0, kt_, qt_)

        # V = x @ Wv^T ([tok, chan], ones col 0); ACT drains EXPs(h0)
        for m in range(KC):
            ps = ps_proj.tile([128, GC], F32, tag="ps", name=f"v{m}")
            for k in range(KC):
                nc.tensor.matmul(
                    ps[:], xt[k][:, m * 128:(m + 1) * 128], wv_sb[k][:],
                    start=(k == 0), stop=(k == KC - 1),
                )
            view = va[m][:].rearrange("p (h e) -> p h e", e=128)
            nc.vector.tensor_copy(
                view[:, :, 64:128], ps[:].rearrange("p (h d) -> p h d", d=64)
            )

        kw[1] = wqk_pool.tile([128, C], BF16, tag="wqk", name="kw1")
        nc.sync.dma_start(kw[1][:], wk_d[1])
        qw[1] = wqk_pool.tile([128, C], BF16, tag="wqk", name="qw1")
        nc.sync.dma_start(qw[1][:], wq_d[1])

        exps_b = emit_s(0, 1, kt_, qt_)

        # ---- main pair loop -----------------------------------------
        # Between EXP-gated PV chains, keep the PE fed with the next
        # pair's projections (pairs 0-2) or warms + Y partials (pair 3).
        yw_sb = {}
        y_ps = {}

        def y_partial(m):
            ps = ps_s.tile([128, 2 * 512], F32, tag="ps", name=f"y{m}")
            for kc in (0, 1, 2):
                for n in range(2):
                    nc.tensor.matmul(
                        ps[:, n * 512:(n + 1) * 512],
                        yw_sb[kc][:, m * 128:(m + 1) * 128],
                        ot[kc][:, n * 512:(n + 1) * 512],
                        start=(kc == 0), stop=False,
                    )
            y_ps[m] = ps

        def y_finish(m):
            ps = y_ps.pop(m, None)
            if ps is None:
                ps = ps_s.tile([128, 2 * 512], F32, tag="ps", name=f"y{m}")
                kcs = (0, 1, 2, 3)
            else:
                kcs = (3,)
            for kc in kcs:
                for n in range(2):
                    nc.tensor.matmul(
                        ps[:, n * 512:(n + 1) * 512],
                        yw_sb[kc][:, m * 128:(m + 1) * 128],
                        ot[kc][:, n * 512:(n + 1) * 512],
                        start=(kc == 0), stop=(kc == KCL - 1),
                    )
            ysb = y_pool.tile([128, T], OBF16, tag="ysb", name=f"ysb{m}")
            nc.scalar.activation(ysb[:], ps[:], Ident, bias=bias_sb[m])
            nc.sync.dma_start(yt_d[m], ysb[:])

        for p in range(KCL):
            a, b = 2 * p, 2 * p + 1
            if p > 0:
                exps_a = emit_s(p, 0, kt_, qt_)
                exps_b = emit_s(p, 1, kt_, qt_)
            # filler 1: next pair's K^T (or warms for the last pair)
            if p < KCL - 1:
                ktn = emit_proj("kt", p + 1, kw[p + 1], kt_pool)
            else:
                emit_warm(8)
            for n in range(2):
                emit_pv(a, n, exps_a)
            # filler 2: next pair's Q^T (or Y partials for the last pair)
            if p < KCL - 1:
                qtn = emit_proj("qt", p + 1, qw[p + 1], qt_pool)
            else:
                y_partial(0)
                y_partial(1)
            for n in range(2):
                emit_pv(b, n, exps_b)
            if p < KCL - 1:
                kt_, qt_ = ktn, qtn
                # prefetch weights for pair p+2 and the Y stage
                if p + 2 < KCL:
                    kw[p + 2] = wqk_pool.tile([128, C], BF16, tag="wqk",
                                              name=f"kw{p + 2}")
                    nc.sync.dma_start(kw[p + 2][:], wk_d[p + 2])
                    qw[p + 2] = wqk_pool.tile([128, C], BF16, tag="wqk",
                                              name=f"qw{p + 2}")
                    nc.sync.dma_start(qw[p + 2][:], wq_d[p + 2])
                for m in (2 * p, 2 * p + 1):
                    if m < KCL:
                        yw = wp_pool.tile([128, C], BF16, tag="wp",
                                          name=f"yw{m}")
                        nc.sync.dma_start(yw[:], wp_d[m])
                        yw_sb[m] = yw

        # ---- output projection --------------------------------------
        emit_warm(4)
        for m in range(8):
            y_finish(m)

    nc.compile()
    return nc


def _get_program():
    global _PROG
    if _PROG is None:
        _PROG = _build_program()
    return _PROG


def _prep_inputs(x, Wqkv, Wproj, bproj):
    """Host-side shard prep: per-core input maps."""
    import ml_dtypes

    bf16 = np.float32 if USE_F32R else ml_dtypes.bfloat16
    x = np.asarray(x, dtype=np.float32)
    Wqkv = np.asarray(Wqkv, dtype=np.float32)
    Wproj = np.asarray(Wproj, dtype=np.float32)
    bproj = np.asarray(bproj, dtype=np.float32)

    def stat_cols(wT_slice):
        # [C, GC] (in chan, local out) -> [KCL, 128, C] stationary tiles:
        # tile[p][part, kc*128+m] = wT_slice[kc*128+part, p*128+m]
        return np.ascontiguousarray(
            wT_slice.reshape(KC, 128, KCL, 128)
            .transpose(2, 1, 0, 3)
            .reshape(KCL, 128, C)
        ).astype(bf16)

    bias = np.ascontiguousarray(bproj.reshape(8, 128).T)
    zbias = np.zeros_like(bias)

    per_g = []
    for g in range(2):
        lo = GC * g
        wq = stat_cols(Wqkv[0 * C + lo:0 * C + lo + GC].T)
        wk = stat_cols(Wqkv[1 * C + lo:1 * C + lo + GC].T)
        wv = np.ascontiguousarray(
            Wqkv[2 * C + lo:2 * C + lo + GC].T.reshape(KC, 128, GC)
        ).astype(bf16)
        # yw[kc][part, m*128+d] = Wproj^T[lo + kc*128 + part, m*128+d]
        wp = np.ascontiguousarray(
            Wproj.T[lo:lo + GC].reshape(KCL, 128, C)
        ).astype(bf16)
        per_g.append((wq, wk, wv, wp))

    in_maps = []
    for i in range(8):
        b, g = divmod(i, 2)
        wq, wk, wv, wp = per_g[g]
        xt = np.ascontiguousarray(x[b].T.reshape(KC, 128, T)).astype(bf16)
        in_maps.append(
            {
                "xt": xt, "wq": wq, "wk": wk, "wv": wv, "wp": wp,
                "bias": bias if g == 0 else zbias,
            }
        )
    return in_maps


def _assemble(results, x_dtype):
    out = np.empty((B, T, C), dtype=np.float32)
    for b in range(B):
        yt = results[2 * b]["yt"].reshape(C, T).astype(np.float32)
        yt = yt + results[2 * b + 1]["yt"].reshape(C, T)
        out[b] = yt.T
    return out.astype(x_dtype, copy=False)


def run(inputs, trace=False, **spmd_kwargs):
    """Shared entry for kernel() and test harnesses (trace for profiling)."""
    from concourse.bass_utils import run_bass_kernel_spmd

    nc = _get_program()
    in_maps = _prep_inputs(**inputs)
    res = run_bass_kernel_spmd(
        nc, in_maps, list(range(8)), trace=trace, **spmd_kwargs
    )
    out = _assemble(res.results, np.asarray(inputs["x"]).dtype)
    return out, res


def kernel(x, Wqkv, Wproj, bproj):
    out, _ = run(dict(x=x, Wqkv=Wqkv, Wproj=Wproj, bproj=bproj))
    return out
